# revision 17
# baseline (speedup 1.0000x reference)
"""Trainium2 Bass kernel for nn_Cell_1_0_35699768164365 (lidar/HSI cross-attention fusion).

Data-parallel over batch across 8 NeuronCores. Math (biases are structurally zero
in this problem; guarded with host fallbacks):
  per b:  scores1 = l_b (Wq^T Wk) l_b^T ;  P1 = softmax(scores1)
          h_emb_b = P1 @ ((l_b Wq^T) * (h_b Wv^T))        (dir 1)
          scores2 = h_b A h_b^T ; l_emb_b = P2 @ ((h Wq^T)*(l Wv^T))  (dir 2)
          emb_b   = Wc[:, :64] @ l_emb_b + Wc[:, 64:] @ h_emb_b
  router: softmax(relu(pooled@Wr1+br1)@Wr2+br2) computed on host (tiny).

Device dataflow per 8-batch block (all T-layout [feature, (b,c)]):
  DMA in -> PE transpose -> projections (batched matmuls) -> per-b scores MMs
  -> exp(ACT) -> group-reduce+recip+normalize (DVE) -> per-b M=P^T Wc_half^T MMs
  -> PE transpose of q*v -> per-b accumulating emb MMs -> evac -> DMA out.
"""

import numpy as np

B = 2048
C = 64
SA = 121
NCORES = 8
B_LOC = B // NCORES      # 256
BLK = 8                  # batch items per block
NBLK_FULL = B_LOC // BLK # 32

_CACHE = {}


def _build(n_blocks):
    import concourse.mybir as mybir
    from concourse import bacc, tile
    from contextlib import ExitStack

    fp32 = mybir.dt.float32
    nc = bacc.Bacc(None, target_bir_lowering=False)

    rows = n_blocks * BLK * C
    WCOLS = 4 * SA + 2 * C   # 612: WQ | WV | AZ | IDN121 | WC1 | WC2
    # xl is pre-transposed and prefixed with the packed weights so block 0's
    # single DMA delivers both (PE matmuls here tolerate only ONE sem wait).
    xl = nc.dram_tensor("xl", [SA, WCOLS + rows], fp32, kind="ExternalInput")
    xh = nc.dram_tensor("xh", [SA, rows], fp32, kind="ExternalInput")
    emb = nc.dram_tensor("emb", [rows, SA], fp32, kind="ExternalOutput")

    ex = ExitStack()
    with tile.TileContext(nc) as tc:
        consts = ex.enter_context(tc.tile_pool(name="consts", bufs=1))
        io_pool = ex.enter_context(tc.tile_pool(name="io", bufs=3))
        sb_pool = ex.enter_context(tc.tile_pool(name="sb", bufs=2))
        tiny = ex.enter_context(tc.tile_pool(name="tiny", bufs=2))
        ps_proj = ex.enter_context(tc.tile_pool(name="ps_proj", bufs=2, space="PSUM"))
        ps_sc = ex.enter_context(tc.tile_pool(name="ps_sc", bufs=1, space="PSUM"))
        ps_m = ex.enter_context(tc.tile_pool(name="ps_m", bufs=1, space="PSUM"))
        ps_qvn = ex.enter_context(tc.tile_pool(name="ps_qvn", bufs=2, space="PSUM"))
        ps_emb = ex.enter_context(tc.tile_pool(name="ps_emb", bufs=2, space="PSUM"))

        COLS = BLK * C            # 512
        wstage = consts.tile([SA, WCOLS], fp32, tag="wstage")
        emb_v = emb.rearrange("(n b c) s -> n c b s", c=C, b=BLK)  # [nblk,64,8,121]

        def wslices(base):
            return (base[0:SA, 0:SA], base[0:SA, SA:2 * SA],
                    base[0:SA, 2 * SA:3 * SA], base[0:SA, 3 * SA:4 * SA],
                    base[0:C, 4 * SA:4 * SA + C],
                    base[0:C, 4 * SA + C:4 * SA + 2 * C])

        for nb in range(n_blocks):
            if nb == 0:
                l0 = io_pool.tile([SA, WCOLS + COLS], fp32, tag="lT0")
                nc.sync.dma_start(out=l0[:, :], in_=xl[:, 0:WCOLS + COLS])
                # stage weights for later blocks; first ACT op -> its sem tick
                # is subsumed by every later PE wait on ACT.
                nc.scalar.copy(wstage[:, :], l0[:, 0:WCOLS])
                lT = l0[:, WCOLS:]
                wq_sb, wv_sb, az_sb, idn_sb, wc1_sb, wc2_sb = wslices(l0)
            else:
                lTt = io_pool.tile([SA, COLS], fp32, tag="lT")
                nc.sync.dma_start(
                    out=lTt[:, :],
                    in_=xl[:, WCOLS + nb * COLS:WCOLS + (nb + 1) * COLS])
                lT = lTt[:, :]
                wq_sb, wv_sb, az_sb, idn_sb, wc1_sb, wc2_sb = wslices(wstage)
            hT = io_pool.tile([SA, COLS], fp32, tag="hT")
            nc.sync.dma_start(out=hT[:, :], in_=xh[:, nb * COLS:(nb + 1) * COLS])

            emb_sb = sb_pool.tile([C, BLK, SA], fp32, tag="emb_sb")

            for d in range(2):
                x = lT if d == 0 else hT[:, :]
                y = hT[:, :] if d == 0 else lT
                wcd = wc2_sb if d == 0 else wc1_sb

                zT = ps_proj.tile([SA, COLS], fp32, tag="proj")
                nc.tensor.matmul(zT[:, :], az_sb, x, start=True, stop=True)
                z_sb = sb_pool.tile([SA, COLS], fp32, tag="z_sb")
                nc.scalar.copy(z_sb[:, :], zT[:, :])

                qT = ps_proj.tile([SA, COLS], fp32, tag="proj")
                vT = ps_proj.tile([SA, COLS], fp32, tag="proj")
                nc.tensor.matmul(qT[:, :], wq_sb, x, start=True, stop=True)
                nc.tensor.matmul(vT[:, :], wv_sb, y, start=True, stop=True)
                v_sb = sb_pool.tile([SA, COLS], fp32, tag="v_sb")
                nc.vector.tensor_copy(v_sb[:, :], vT[:, :])
                qv = sb_pool.tile([SA, COLS], fp32, tag="qv")
                nc.vector.tensor_mul(qv[:, :], qT[:, :], v_sb[:, :])

                scores = ps_sc.tile([C, COLS], fp32, tag="sc")
                for j in range(BLK):
                    sl = slice(j * C, (j + 1) * C)
                    nc.tensor.matmul(scores[:, sl], z_sb[:, sl], x[:, sl],
                                     start=True, stop=True)

                E = sb_pool.tile([C, COLS], fp32, tag="E")
                nc.scalar.activation(E[:, :], scores[:, :],
                                     mybir.ActivationFunctionType.Exp)
                Zr = tiny.tile([C, BLK], fp32, tag="Zr")
                nc.vector.tensor_reduce(
                    Zr[:, :], E[:, :].rearrange("p (b g) -> p b g", g=C),
                    axis=mybir.AxisListType.X, op=mybir.AluOpType.add)
                rZ = tiny.tile([C, BLK], fp32, tag="rZ")
                nc.vector.reciprocal(rZ[:, :], Zr[:, :])
                P = sb_pool.tile([C, COLS], fp32, tag="P")
                nc.vector.tensor_tensor(
                    out=P[:, :].rearrange("p (b g) -> p b g", g=C),
                    in0=E[:, :].rearrange("p (b g) -> p b g", g=C),
                    in1=rZ[:, :].unsqueeze(2).broadcast_to((C, BLK, C)),
                    op=mybir.AluOpType.mult)

                M_ps = ps_m.tile([C, COLS], fp32, tag="m")
                for j in range(BLK):
                    sl = slice(j * C, (j + 1) * C)
                    nc.tensor.matmul(M_ps[:, sl], P[:, sl], wcd,
                                     start=True, stop=True)
                M_sb = sb_pool.tile([C, COLS], fp32, tag="M_sb")
                nc.vector.tensor_copy(M_sb[:, :], M_ps[:, :])

                qvn_ps0 = ps_qvn.tile([C, 4 * SA], fp32, tag="qvn")
                qvn_ps1 = ps_qvn.tile([C, 4 * SA], fp32, tag="qvn")
                for j in range(BLK):
                    qp = qvn_ps0 if j < 4 else qvn_ps1
                    nc.tensor.transpose(
                        qp[:, (j % 4) * SA:(j % 4 + 1) * SA],
                        qv[:, j * C:(j + 1) * C], idn_sb)
                qvn = sb_pool.tile([C, BLK * SA], fp32, tag="qvnsb")
                nc.vector.tensor_copy(qvn[:, 0:4 * SA], qvn_ps0[:, :])
                nc.vector.tensor_copy(qvn[:, 4 * SA:8 * SA], qvn_ps1[:, :])

                emb_ps0 = ps_emb.tile([C, 4 * SA], fp32, tag="embp")
                emb_ps1 = ps_emb.tile([C, 4 * SA], fp32, tag="embp")
                for j in range(BLK):
                    ep = emb_ps0 if j < 4 else emb_ps1
                    jj = j % 4
                    nc.tensor.matmul(
                        ep[:, jj * SA:(jj + 1) * SA],
                        M_sb[:, j * C:(j + 1) * C],
                        qvn[:, j * SA:(j + 1) * SA],
                        start=True, stop=True)
                half0 = emb_sb[:, 0:4, :].rearrange("p b s -> p (b s)")
                half1 = emb_sb[:, 4:8, :].rearrange("p b s -> p (b s)")
                if d == 0:
                    nc.scalar.copy(half0, emb_ps0[:, :])
                    nc.scalar.copy(half1, emb_ps1[:, :])
                else:
                    nc.vector.tensor_add(half0, emb_ps0[:, :], half0)
                    nc.vector.tensor_add(half1, emb_ps1[:, :], half1)

            nc.sync.dma_start(out=emb_v[nb], in_=emb_sb[:, :, :])

        ex.close()

    nc.compile()
    return nc


def _reference_numpy(lidar, hsi, Wq, bq, Wk, bk, Wv, bv, Wr1, br1, Wr2, br2, Wc, bc):
    l = lidar.reshape(B, C, SA).astype(np.float64)
    h = hsi.reshape(B, C, SA).astype(np.float64)

    def cross(x, y):
        q = x @ Wq.T + bq
        k = x @ Wk.T + bk
        v = y @ Wv.T + bv
        s = np.einsum("bhs,bgs->bhg", q, k)
        s -= s.max(-1, keepdims=True)
        e = np.exp(s)
        p = e / e.sum(-1, keepdims=True)
        return np.einsum("bhg,bgs->bhs", p, q * v)

    h_emb = cross(l, h)
    l_emb = cross(h, l)
    cat = np.concatenate([l_emb, h_emb], 1)
    out = np.einsum("oc,bcs->bos", Wc.astype(np.float64), cat) + bc[None, :, None]
    return out.reshape(B, C, 11, 11).astype(np.float32)


def _router_host(lidar, hsi, Wr1, br1, Wr2, br2):
    pl = lidar.reshape(B, C, SA).mean(axis=2)
    ph = hsi.reshape(B, C, SA).mean(axis=2)
    pooled = np.concatenate([pl, ph], axis=1)
    hdn = np.maximum(pooled @ Wr1 + br1, 0.0)
    logits = hdn @ Wr2 + br2
    m = logits.max(-1, keepdims=True)
    e = np.exp(logits - m)
    return (e / e.sum(-1, keepdims=True)).astype(np.float32)


def kernel(lidar, hsi, Wq, bq, Wk, bk, Wv, bv, Wr1, br1, Wr2, br2, Wc, bc):
    lidar = np.asarray(lidar, np.float32)
    hsi = np.asarray(hsi, np.float32)
    args = [np.asarray(a, np.float32) for a in
            (Wq, bq, Wk, bk, Wv, bv, Wr1, br1, Wr2, br2, Wc, bc)]
    Wq, bq, Wk, bk, Wv, bv, Wr1, br1, Wr2, br2, Wc, bc = args

    path_prob = _router_host(lidar, hsi, Wr1, br1, Wr2, br2)

    if np.any(bq) or np.any(bk) or np.any(bv):
        emb_full = _reference_numpy(lidar, hsi, Wq, bq, Wk, bk, Wv, bv,
                                    Wr1, br1, Wr2, br2, Wc, bc)
        return (emb_full, path_prob)

    from concourse.bass_utils import run_bass_kernel_spmd

    if "nc" not in _CACHE:
        _CACHE["nc"] = _build(NBLK_FULL)
    nc = _CACHE["nc"]

    WCOLS = 4 * SA + 2 * C
    Wh = np.zeros((SA, WCOLS), np.float32)
    Wh[:, 0:SA] = Wq.T
    Wh[:, SA:2 * SA] = Wv.T
    Wh[:, 2 * SA:3 * SA] = Wq.T @ Wk
    Wh[:, 3 * SA:4 * SA] = np.eye(SA, dtype=np.float32)
    Wh[0:C, 4 * SA:4 * SA + C] = Wc[:, :C].T
    Wh[0:C, 4 * SA + C:4 * SA + 2 * C] = Wc[:, C:].T

    l_flat = lidar.reshape(B * C, SA)
    h_flat = hsi.reshape(B * C, SA)
    in_maps = []
    for i in range(NCORES):
        r0 = i * B_LOC * C
        r1 = (i + 1) * B_LOC * C
        in_maps.append({
            "xl": np.ascontiguousarray(
                np.concatenate([Wh, l_flat[r0:r1].T], axis=1)),
            "xh": np.ascontiguousarray(h_flat[r0:r1].T),
        })

    res = run_bass_kernel_spmd(nc, in_maps, list(range(NCORES)))
    emb_full = np.concatenate(
        [res.results[i]["emb"].reshape(B_LOC, C, 11, 11) for i in range(NCORES)],
        axis=0)
    if np.any(bc):
        emb_full = emb_full + bc[None, :, None, None].astype(np.float32)
    return (emb_full, path_prob)
